# revision 22
# baseline (speedup 1.0000x reference)
"""Bass/Trainium2 kernel for nn_Channel_attention (bottom-16 channel gather).

reference semantics (per sample b):
    weight = mean(x[b], axis=(H, W))           # [C]
    idx    = argsort(weight)[:16]              # ascending pooled value
    out[b] = x[b, idx]                         # [16, H, W]

Strategy: pure data parallel, B=16 sharded 2 samples per core over 8 cores.
Per core (x shard viewed as [512, 16384] = [(sample, channel), H*W]):

  The whole 32 MiB must stream HBM->SBUF once for the pooled means, so the
  kernel is paced by the load stream (~425 GB/s with deep queue backlog).
  Two changes versus the naive pipeline keep the stream at full rate and
  shorten the tail:

  1. Reduction work is split between the Vector (DVE) and Scalar (ACT)
     engines (DVE tensor_reduce / ACT activation-with-accum_out), so
     neither engine saturates and pool-buffer recycling never throttles
     DMA issue.  All chunk loads are emitted before their reduces so the
     issue streams on sync/scalar never head-of-line block on compute.

  2. Channel selection is computed via ranks, not max8 rounds:
     rank[c] = #{c' : w[c'] < w[c]} obtained from one replicated-w matmul
     plus one fused DVE compare-and-accumulate per 128-channel half.  The
     rank IS the output row, so the last-arriving sample's 16 channels are
     scattered straight from its resident SBUF tiles with two OOB-skipped
     SWDGE indirect DMAs (no gather re-read on the critical tail).

  Sample A (rows 0..255) streams first through a rotating pool; its 16
  channels are re-fetched mid-run with one full-width [128, 2048] indirect
  gather and stored while sample B still streams.  Sample B (rows
  256..511) loads second into two resident [128, 16384] tiles with
  shrinking tail chunks; after its last 0.25 MiB chunk lands, only the
  rank chain (~2 us) and the two scatters sit on the critical path.
  Scatters go to "y" (half 0, rows 16..31) and "y2" (half 1) so no
  write-write dependency serializes them; the host merges rows using the
  per-channel ranks streamed out in "rk".
"""

import sys

if "/opt/trn_rl_repo" not in sys.path:
    sys.path.insert(0, "/opt/trn_rl_repo")

import numpy as np

from concourse import bacc, mybir, tile
from concourse.bass import IndirectOffsetOnAxis
from concourse.bass_utils import run_bass_kernel_spmd
from concourse.masks import make_identity

N_CORES = 8
B, C, H, W = 16, 256, 128, 128
K = 16
BPC = B // N_CORES          # samples per core = 2
E = H * W                   # 16384 elems per channel
ROWS = BPC * C              # 512 channel rows per core
GU = 8                      # gather sub-rows per channel (8 x 8KiB)

f32 = mybir.dt.float32
i32 = mybir.dt.int32
X = mybir.AxisListType.X
Alu = mybir.AluOpType
Act = mybir.ActivationFunctionType

# sample A: 16 chunks of 2048 cols (8 per half), pooled
A_NCH = 8
A_CW = 2048
# sample B: resident; (half, col_off, width) in emission order, tail shrinks
B_CHUNKS = [
    (0, 0, 4096), (1, 0, 4096), (0, 4096, 4096), (1, 4096, 4096),
    (0, 8192, 4096), (1, 8192, 4096), (0, 12288, 4096), (1, 12288, 2048),
    (1, 14336, 1024), (1, 15360, 512), (1, 15872, 512),
]
B_H0_CHUNKS = 4
B_H1_CHUNKS = 7

_cache = {}


class _FastExitTileContext(tile.TileContext):
    """TileContext whose epilogue skips the per-semaphore clear pass.

    The standard exit drains DMA, barriers, then zeroes every allocated
    semaphore one by one (~250 sems x ~30 ns = ~7 us on the critical
    path).  That clear only matters when another kernel follows in the
    same loaded program; this kernel is single-shot per runtime session,
    so we keep the drain + one all-engine barrier and drop the clears.
    """

    def _drain_and_barrier(self, tick_clock, wait_clock):
        from concourse.vector_clock import ScopedClock

        drain_inst = self.nc.sync.drain()
        wait_clock.add_sem_waits(
            drain_inst.ins, ScopedClock({None: tick_clock.global_clock})
        )
        self.nc.all_engine_barrier()
        popped = self.nc._tile_sem_poison_stack.pop()
        assert popped is self._sem_poison

def _build():
    nc = bacc.Bacc("TRN2", target_bir_lowering=False, debug=False,
                   num_devices=N_CORES)
    x_d = nc.dram_tensor("x", [ROWS, E], f32, kind="ExternalInput")
    y_d = nc.dram_tensor("y", [2 * K, E], f32, kind="ExternalOutput")
    y2_d = nc.dram_tensor("y2", [K, E], f32, kind="ExternalOutput")
    rk_d = nc.dram_tensor("rk", [128, 2], f32, kind="ExternalOutput")

    with _FastExitTileContext(nc) as tc:
        with (
            tc.tile_pool(name="load", bufs=8) as load_pool,
            tc.tile_pool(name="small", bufs=1) as small,
            tc.tile_pool(name="psum", bufs=1, space="PSUM") as psum,
        ):
            # ---------------- constants (no deps) ----------------
            ident = small.tile([128, 128], f32)
            make_identity(nc, ident[:])

            ones1 = small.tile([1, 128], f32)
            nc.vector.memset(ones1[:], 1.0)

            # row16[p, r] = r  (f32)
            row16_i = small.tile([128, K], i32)
            nc.gpsimd.iota(out=row16_i[:], pattern=[[1, K]], base=0,
                           channel_multiplier=0)
            row16 = small.tile([128, K], f32)
            nc.vector.tensor_copy(row16[:], row16_i[:])

            # oh16_8[j, p] = (p>>3 == j), j in 0..15: expands 16 ranks to
            # 128 gather rows
            r8_i = small.tile([K, 128], i32)
            nc.gpsimd.iota(out=r8_i[:], pattern=[[1, 128]], base=0,
                           channel_multiplier=0)
            nc.vector.tensor_scalar(out=r8_i[:], in0=r8_i[:], scalar1=3,
                                    scalar2=None, op0=Alu.arith_shift_right)
            r8_f = small.tile([K, 128], f32)
            nc.vector.tensor_copy(r8_f[:], r8_i[:])
            col16_i = small.tile([K, 1], i32)
            nc.gpsimd.iota(out=col16_i[:], pattern=[[1, 1]], base=0,
                           channel_multiplier=1)
            col16 = small.tile([K, 1], f32)
            nc.vector.tensor_copy(col16[:], col16_i[:])
            oh16_8 = small.tile([K, 128], f32)
            nc.vector.tensor_scalar(out=oh16_8[:], in0=r8_f[:],
                                    scalar1=col16[:], scalar2=None,
                                    op0=Alu.is_equal)

            # channel iota per half: ci[h][p] = p + 128h (f32)
            ci_i = small.tile([128, 1], i32)
            nc.gpsimd.iota(out=ci_i[:], pattern=[[1, 1]], base=0,
                           channel_multiplier=1)
            ci0 = small.tile([128, 1], f32)
            nc.vector.tensor_copy(ci0[:], ci_i[:])
            ci1 = small.tile([128, 1], f32)
            nc.vector.tensor_scalar(out=ci1[:], in0=ci0[:], scalar1=128.0,
                                    scalar2=None, op0=Alu.add)
            ci = [ci0, ci1]

            # a7[p] = p & 7 (f32), gather sub-row offset
            a7_i = small.tile([128, 1], i32)
            nc.gpsimd.iota(out=a7_i[:], pattern=[[1, 1]], base=0,
                           channel_multiplier=1)
            nc.vector.tensor_scalar(out=a7_i[:], in0=a7_i[:], scalar1=GU - 1,
                                    scalar2=None, op0=Alu.bitwise_and)
            a7 = small.tile([128, 1], f32)
            nc.vector.tensor_copy(a7[:], a7_i[:])

            xg8 = x_d[:].rearrange("r (u e) -> (r u) e", u=GU)
            dma_engines = [nc.sync, nc.scalar]
            red_engines = ["v", "s"]
            state = {"n_dma": 0, "n_red": 0}

            def emit_load(dst, src):
                eng = dma_engines[state["n_dma"] % 2]
                state["n_dma"] += 1
                eng.dma_start(out=dst, in_=src)

            def emit_reduce(partials_col, tile_ap):
                if red_engines[state["n_red"] % 2] == "v":
                    nc.vector.reduce_sum(out=partials_col, in_=tile_ap,
                                         axis=X)
                else:
                    nc.scalar.activation(out=tile_ap, in_=tile_ap,
                                         func=Act.Copy,
                                         accum_out=partials_col)
                state["n_red"] += 1

            def rank_chain(tag, sums, psum_w, w_row, psum_W, cmp, rank, h):
                """sums[:, h] -> w row (transpose) -> replicate -> fused
                compare+count => rank[:, h].  Emit per half; the compare
                needs BOTH halves' psum_W columns written."""
                nc.tensor.matmul(out=psum_w[:, h * 128:(h + 1) * 128],
                                 lhsT=sums[:, h:h + 1], rhs=ident[:],
                                 start=True, stop=True)
                nc.vector.tensor_copy(w_row[:, h * 128:(h + 1) * 128],
                                      psum_w[:, h * 128:(h + 1) * 128])
                nc.tensor.matmul(out=psum_W[:, h * 128:(h + 1) * 128],
                                 lhsT=ones1[:],
                                 rhs=w_row[:, h * 128:(h + 1) * 128],
                                 start=True, stop=True)

            # ================ sample A: pooled loads =================
            a_tiles = []
            for j in range(A_NCH):
                for h in range(2):
                    t = load_pool.tile([128, A_CW], f32)
                    emit_load(t[:], x_d[h * 128:(h + 1) * 128,
                                        j * A_CW:(j + 1) * A_CW])
                    a_tiles.append((t, h * A_NCH + j))

            # ================ sample A: reduces ======================
            partialsA = small.tile([128, 2 * A_NCH], f32, tag="pA")
            for t, col in a_tiles:
                emit_reduce(partialsA[:, col:col + 1], t[:])

            # ================ sample A: select + gather ==============
            sumsA = small.tile([128, 2], f32, tag="sumsA")
            psum_wA = psum.tile([1, C], f32, tag="pswA")
            w_rowA = small.tile([1, C], f32, tag="wrowA")
            psum_WA = psum.tile([128, C], f32, tag="psWA")
            cmp = small.tile([128, C], f32, tag="cmp")
            rankA = small.tile([128, 2], f32, tag="rankA")
            for h in range(2):
                nc.vector.reduce_sum(
                    out=sumsA[:, h:h + 1],
                    in_=partialsA[:, h * A_NCH:(h + 1) * A_NCH], axis=X)
                rank_chain("A", sumsA, psum_wA, w_rowA, psum_WA, cmp,
                           rankA, h)
            for h in range(2):
                nc.vector.tensor_scalar(out=cmp[:], in0=psum_WA[:],
                                        scalar1=sumsA[:, h:h + 1],
                                        scalar2=None, op0=Alu.is_lt,
                                        op1=Alu.add,
                                        accum_out=rankA[:, h:h + 1])

            # channel index per rank: chan16[r] = sum_p p_global * (rank==r)
            psum_c16 = psum.tile([K, 1], f32, tag="psc16")
            for h in range(2):
                ohh = small.tile([128, K], f32, tag=f"ohA{h}")
                nc.vector.tensor_scalar(out=ohh[:], in0=row16[:],
                                        scalar1=rankA[:, h:h + 1],
                                        scalar2=None, op0=Alu.is_equal)
                nc.tensor.matmul(out=psum_c16[:], lhsT=ohh[:], rhs=ci[h][:],
                                 start=(h == 0), stop=(h == 1))
            c16 = small.tile([K, 1], f32, tag="c16")
            nc.vector.tensor_copy(c16[:], psum_c16[:])
            # expand to 128 gather rows: grow[p] = chan16[p>>3]*8 + (p&7)
            psum_e = psum.tile([128, 1], f32, tag="pse")
            nc.tensor.matmul(out=psum_e[:], lhsT=oh16_8[:], rhs=c16[:],
                             start=True, stop=True)
            grow_f = small.tile([128, 1], f32, tag="growf")
            nc.vector.tensor_scalar(out=grow_f[:], in0=psum_e[:],
                                    scalar1=float(GU), scalar2=None,
                                    op0=Alu.mult)
            grow_i = small.tile([128, 1], i32, tag="growi")
            nc.vector.tensor_tensor(out=grow_i[:], in0=grow_f[:], in1=a7[:],
                                    op=Alu.add)
            gA = small.tile([128, E // GU], f32, tag="gA")
            nc.gpsimd.indirect_dma_start(
                out=gA[:], out_offset=None, in_=xg8,
                in_offset=IndirectOffsetOnAxis(ap=grow_i[:], axis=0))

            # ================ sample B: resident loads ===============
            big0 = small.tile([128, E], f32, tag="big0")
            big1 = small.tile([128, E], f32, tag="big1")
            big = [big0, big1]
            for h, off, cw in B_CHUNKS:
                emit_load(big[h][:, off:off + cw],
                          x_d[C + h * 128:C + (h + 1) * 128, off:off + cw])

            # sample A's gathered channels stored while B streams.  The
            # store rides the gpsimd (SWDGE) queue right behind the gather
            # so the sync/scalar queues carry nothing but loads - a store
            # frozen mid-stream would head-of-line block load issue until
            # the gather lands.
            yA8 = y_d[0:K].rearrange("r (u e) -> (r u) e", u=GU)
            nc.gpsimd.dma_start(out=yA8, in_=gA[:])

            # ================ sample B: reduces ======================
            partialsB = small.tile([128, B_H0_CHUNKS + B_H1_CHUNKS], f32,
                                   tag="pB")
            ncol = [0, 0]
            sumsB = small.tile([128, 2], f32, tag="sumsB")
            psum_wB = psum.tile([1, C], f32, tag="pswB")
            w_rowB = small.tile([1, C], f32, tag="wrowB")
            psum_WB = psum.tile([128, C], f32, tag="psWB")
            rankB = small.tile([128, 2], f32, tag="rankB")
            for h, off, cw in B_CHUNKS:
                col = (0 if h == 0 else B_H0_CHUNKS) + ncol[h]
                ncol[h] += 1
                emit_reduce(partialsB[:, col:col + 1], big[h][:, off:off + cw])
                if h == 0 and ncol[0] == B_H0_CHUNKS:
                    # half 0 fully reduced well before the tail: run its
                    # share of the rank chain early
                    nc.vector.reduce_sum(out=sumsB[:, 0:1],
                                         in_=partialsB[:, 0:B_H0_CHUNKS],
                                         axis=X)
                    rank_chain("B", sumsB, psum_wB, w_rowB, psum_WB, None,
                               rankB, 0)

            # ================ sample B: tail select + scatter ========
            nc.vector.reduce_sum(
                out=sumsB[:, 1:2],
                in_=partialsB[:, B_H0_CHUNKS:B_H0_CHUNKS + B_H1_CHUNKS],
                axis=X)
            rank_chain("B", sumsB, psum_wB, w_rowB, psum_WB, None, rankB, 1)
            for h in range(2):
                nc.vector.tensor_scalar(out=cmp[:], in0=psum_WB[:],
                                        scalar1=sumsB[:, h:h + 1],
                                        scalar2=None, op0=Alu.is_lt,
                                        op1=Alu.add,
                                        accum_out=rankB[:, h:h + 1])
            nc.scalar.dma_start(out=rk_d[:], in_=rankB[:])

            # offsets: half0 -> y rows 16+rank (OOB skipped past 31),
            # half1 -> y2 rows rank (OOB skipped past 15)
            offs0 = small.tile([128, 1], i32, tag="offs0")
            nc.vector.tensor_scalar(out=offs0[:], in0=rankB[:, 0:1],
                                    scalar1=float(K), scalar2=None,
                                    op0=Alu.add)
            offs1 = small.tile([128, 1], i32, tag="offs1")
            nc.vector.tensor_copy(offs1[:], rankB[:, 1:2])
            nc.gpsimd.indirect_dma_start(
                out=y_d[:],
                out_offset=IndirectOffsetOnAxis(ap=offs0[:], axis=0),
                in_=big0[:], in_offset=None,
                bounds_check=2 * K - 1, oob_is_err=False)
            nc.gpsimd.indirect_dma_start(
                out=y2_d[:],
                out_offset=IndirectOffsetOnAxis(ap=offs1[:], axis=0),
                in_=big1[:], in_offset=None,
                bounds_check=K - 1, oob_is_err=False)

    nc.compile()
    return nc


def get_nc():
    if "nc" not in _cache:
        _cache["nc"] = _build()
    return _cache["nc"]


def make_in_maps(x: np.ndarray) -> list[dict[str, np.ndarray]]:
    x = np.ascontiguousarray(np.asarray(x, dtype=np.float32))
    assert x.shape == (B, C, H, W)
    return [{"x": x[c * BPC:(c + 1) * BPC].reshape(ROWS, E)}
            for c in range(N_CORES)]


def assemble(results: list[dict[str, np.ndarray]]) -> np.ndarray:
    out = np.empty((B, K, H, W), dtype=np.float32)
    for c in range(N_CORES):
        y = results[c]["y"].reshape(2 * K, H, W)
        y2 = results[c]["y2"].reshape(K, H, W)
        rk1 = results[c]["rk"][:, 1].astype(np.int64)   # half-1 ranks
        out[c * BPC] = y[0:K]
        sb = y[K:2 * K].copy()
        for r in rk1[(rk1 >= 0) & (rk1 < K)]:
            sb[r] = y2[r]
        out[c * BPC + 1] = sb
    return out


def kernel(x: np.ndarray) -> np.ndarray:
    nc = get_nc()
    res = run_bass_kernel_spmd(nc, make_in_maps(x), list(range(N_CORES)))
    return assemble(res.results)
